# revision 21
# baseline (speedup 1.0000x reference)
"""Multi-head causal attention (B=4, T=2048, D=512, H=8) on 8 TRN2 NeuronCores.

Sharding: core c handles batch b = c//2 and head-group hg = c%2 (4 heads,
256 output dims).  No collectives needed — 8 fully independent problems.

Per-core algorithm (matmul inputs bf16, O^T accumulation f32 in PSUM):
  - host passes x^T (D,T) and W^T slices (D, 256) in bf16 + a [128,128]
    triangular causal mask
  - Q^T,K^T projections:  qT[dh2,T] = W2h @ xT, two heads stacked per tile
    (head 2g at partitions 0-63, head 2g+1 at partitions 64-127)
  - V projection into augmented-V tiles [k-tile 128, 65] (ones column
    appended -> the O^T matmul also produces the softmax denominator row)
  - flash-style over head-PAIRS: for each (q-block, pair g), per k-tile the
    two heads' score matmuls S^T[k,q] = K^T.T @ Q^T are row-tiled
    (tile_position (0,0) and (64,0) via base_partition auto-derive) and run
    CONCURRENTLY on the PE — K=64 each, so the pair streams in the time of
    one matmul.  exp via ACT (scale=1/8 folded; no max subtraction:
    |scores| < ~4), causal via per-k-tile width restriction + triangle-mask
    multiply on the boundary block.
  - O^T accumulated in PSUM over k-tiles (start/stop groups), software
    pipelined one batch deep: AV(kt-1) is emitted after scores(kt) so the
    PE never stalls on exp(kt).
  - epilogue per unit: single DVE cast of [65, 2, 512] O^T+denominator to
    bf16, DMA'd out UNNORMALIZED; the host divides by the denominator row
    and transposes to natural layout (removes all PE transposes and DVE
    normalize work from the device).

Scheduling (program order == Tile priority): warm-up matmul bursts cover
the x^T DMA shadow (HAM), minimal projection prologue (K g0 chunk 0 +
8-warm bridge + Q g0 chunk 3 + V tile 0), then units (qb DESCENDING,
g=0 then 1) with the remaining projection/V units woven between batches
as PE filler in arrival order.  x^T is DMA'd as 16 column-block pieces in
consumption order across the SP/ACT/gpsimd DGE queues; outputs stream per
unit on the SP queue (last unit split SP/ACT to shorten the tail).
"""

import numpy as np
import ml_dtypes

T = 2048
D = 512
HG = 4  # heads per core
DH = 64
OUTW = HG * DH  # 256
QB = 512  # q block (columns of S^T tiles)
NQB = T // QB  # 4
NKT = T // 128  # 16 k-tiles
N_CORES = 8

_CACHE = {}


def _build_nc():
    import concourse.bacc as bacc
    import concourse.tile as tile
    import concourse.mybir as mybir
    from contextlib import ExitStack

    fp32 = mybir.dt.float32
    bf16 = mybir.dt.bfloat16
    EXP = mybir.ActivationFunctionType.Exp

    nc = bacc.Bacc(None, target_bir_lowering=False)

    xt_d = nc.declare_dram_parameter("xt", [D, T], bf16, isOutput=False)
    wqt_d = nc.declare_dram_parameter("wqt", [D, OUTW], bf16, isOutput=False)
    wkt_d = nc.declare_dram_parameter("wkt", [D, OUTW], bf16, isOutput=False)
    wvt_d = nc.declare_dram_parameter("wvt", [D, OUTW], bf16, isOutput=False)
    cmask_d = nc.declare_dram_parameter("cmask", [128, 128], bf16, isOutput=False)
    # unnormalized O^T + denominator row: [head, 65, qb, 512]
    out_d = nc.declare_dram_parameter("out", [HG, 65, NQB, QB], bf16, isOutput=True)

    with tile.TileContext(nc) as tc, ExitStack() as ctx:
        const = ctx.enter_context(tc.tile_pool(name="const", bufs=1))
        ps_s = ctx.enter_context(tc.tile_pool(name="ps_s", bufs=2, space="PSUM"))
        pt_pool = ctx.enter_context(tc.tile_pool(name="pt", bufs=4))
        osb_pool = ctx.enter_context(tc.tile_pool(name="osb", bufs=2))

        # ---- input loads ----
        # Each DMA trigger pays ~0.7us fixed queue time, so inputs load as
        # few, large pieces: each weight matrix as ONE 256KB DMA (3D AP,
        # 2KB/partition) and x^T as 8 chunk-pair x q-block pieces, spread
        # over the three DGE queues (per-queue BW ~75GB/s) in ascending
        # consumption order (block 0 gates the prologue projections).
        # The scalar (ACT) queue only carries pieces that finish BEFORE the
        # first exp, so triggers never steal ACT time from the exp stream.
        xTb2 = [[const.tile([128, 2, QB], bf16, tag=f"xT{cc}_{b}",
                            name=f"xT{cc}_{b}")
                 for b in range(4)] for cc in range(2)]
        wkT = const.tile([128, 4, OUTW], bf16, name="wkT")
        wqT = const.tile([128, 4, OUTW], bf16, name="wqT")
        wvT = const.tile([128, 4, OUTW], bf16, name="wvT")
        mask_sb = const.tile([128, 128], bf16, name="mask_sb")

        def ld_w(eng, wt, dram):
            eng.dma_start(
                out=wt[:], in_=dram[:, :].rearrange("(c p) w -> p c w", p=128)
            )

        def ld_x(eng, cc, b):
            eng.dma_start(
                out=xTb2[cc][b][:],
                in_=xt_d[cc * 256:(cc + 1) * 256,
                         b * QB:(b + 1) * QB].rearrange(
                             "(c p) t -> p c t", p=128),
            )

        ld_w(nc.sync, wkT, wkt_d)
        ld_w(nc.sync, wqT, wqt_d)
        ld_x(nc.sync, 0, 1)
        ld_x(nc.sync, 0, 3)

        ld_x(nc.scalar, 0, 0)
        nc.scalar.dma_start(out=mask_sb[:], in_=cmask_d[:])
        ld_w(nc.scalar, wvT, wvt_d)

        # slow SWDGE triggers (~1us each) carry the remainder
        ld_x(nc.gpsimd, 1, 0)
        ld_x(nc.gpsimd, 1, 1)
        ld_x(nc.gpsimd, 0, 2)
        ld_x(nc.gpsimd, 1, 2)
        ld_x(nc.gpsimd, 1, 3)

        # ---- HAM warm-up burst ----
        # The PE clock needs a fully-busy window to ramp.  Burn the x^T DMA
        # shadow with dense dummy matmuls so real work starts warm.
        warm_w = const.tile([128, 128], bf16, name="warm_w")
        warm_x = const.tile([128, QB], bf16, name="warm_x")
        nc.vector.memset(warm_w[:], 0.5)
        nc.vector.memset(warm_x[:], 0.5)
        # 12 warms ≈ the ~6.5us wk/wq/x-block-0 DMA window (cold 512ns each,
        # ~256ns once the ramp trips mid-burst) — sized so the PE never idles
        # a full MID window before the first projection.
        warm_ps = ps_s.tile([128, QB], fp32, tag="ps", name="warm_ps")
        for _ in range(12):
            nc.tensor.matmul(warm_ps[:], warm_w[:], warm_x[:], start=True, stop=True)

        # ---- persistent SBUF tensors ----
        qT = [const.tile([128, T], bf16, tag=f"qT{g}", name=f"qT{g}") for g in range(2)]
        kT = [const.tile([128, T], bf16, tag=f"kT{g}", name=f"kT{g}") for g in range(2)]
        vaug = const.tile([128, NKT, HG, 65], bf16, name="vaug")
        nc.vector.memset(vaug[:, :, :, 64:65], 1.0)

        def proj_qk(dst, wt, g, qb4):
            ps = ps_s.tile([128, QB], fp32, tag="ps", name="ps")
            for c in range(4):
                nc.tensor.matmul(
                    ps[:],
                    wt[:, c, g * 128:(g + 1) * 128],
                    xTb2[c // 2][qb4][:, c % 2, :],
                    start=(c == 0),
                    stop=(c == 3),
                )
            nc.vector.tensor_copy(dst[g][:, qb4 * QB:(qb4 + 1) * QB], ps[:])

        def proj_v(tt):
            ps = ps_s.tile([128, OUTW], fp32, tag="ps", name="ps")
            for c in range(4):
                nc.tensor.matmul(
                    ps[:],
                    xTb2[c // 2][tt // 4][
                        :, c % 2, (tt % 4) * 128:(tt % 4 + 1) * 128],
                    wvT[:, c, 0:OUTW],
                    start=(c == 0),
                    stop=(c == 3),
                )
            nc.vector.tensor_copy(
                vaug[:, tt, :, 0:64],
                ps[:].rearrange("p (h d) -> p h d", h=HG),
            )

        def attn_unit(qb, g, fillers=None, prefill=None, last=False):
            """One (q-block, head-pair) attention unit.  The two heads'
            score matmuls per k-tile are row-tiled (partitions 0-63 /
            64-127) and run concurrently on the PE.  AV(kt-1) is emitted
            after scores(kt) (1-deep software pipeline) so the PE works
            through the exp latency.  fillers[j] = list of closures run
            as PE filler after batch j's scores."""
            nkt = qb * 4 + 4
            # prefill runs BEFORE the first scores: at a unit boundary the
            # first scores block on the st-slot WAR until ACT drains the
            # previous unit's diagonal-exp backlog, and the in-order PE
            # queue would stall fillers emitted behind them.
            if prefill:
                for f in prefill:
                    f()
            ot = ps_s.tile([128, 2, QB], fp32, tag="ot", bufs=1, name="ot")
            pend = []  # (kt, pt, q0, width) awaiting AV matmuls (lag 2)

            def emit_av(kt, pt, q0, width):
                for i in range(2):
                    nc.tensor.matmul(
                        ot[0:65, i, q0:q0 + width],
                        vaug[:, kt, 2 * g + i, :],
                        pt[:, i, q0:q0 + width],
                        start=(kt == 0),
                        stop=(kt == nkt - 1),
                    )

            for kt in range(nkt):
                diag = kt >= qb * 4
                q0 = (kt - qb * 4) * 128 if diag else 0
                width = QB - q0
                st = ps_s.tile([128, 2, QB], fp32, tag="st", name="st")
                for i in range(2):
                    nc.tensor.matmul(
                        st[:, i, q0:q0 + width],
                        kT[g][64 * i:64 * i + 64, kt * 128:(kt + 1) * 128],
                        qT[g][64 * i:64 * i + 64,
                              qb * QB + q0:qb * QB + q0 + width],
                        start=True,
                        stop=True,
                    )
                if fillers:
                    for f in fillers.get(kt, ()):
                        f()
                # AV lags 2 batches so the unit's first AV (which waits on
                # the previous unit's ot WAR) issues behind two score pairs.
                if len(pend) == 2:
                    emit_av(*pend.pop(0))
                pt = pt_pool.tile([128, 2, QB], bf16, tag="pt", name="pt")
                nc.scalar.activation(
                    pt[:, :, q0:q0 + width], st[:, :, q0:q0 + width],
                    func=EXP, scale=0.125,
                )
                if diag:
                    for i in range(2):
                        nc.vector.tensor_mul(
                            pt[:, i, q0:q0 + 128], pt[:, i, q0:q0 + 128],
                            mask_sb[:],
                        )
                pend.append((kt, pt, q0, width))
            for p in pend:
                emit_av(*p)

            # epilogue: cast O^T + denominator to bf16, stream out
            # unnormalized (host divides + transposes).  h0 cast on DVE,
            # h1 on ACT (idle at unit boundaries; Copy is in every table
            # set, so no table reload), so the ot PSUM slot frees in one
            # cast-time and the two output DMAs ride different queues.
            osb = osb_pool.tile([65, 2, QB], bf16, tag="osb", name="osb")
            nc.vector.tensor_copy(osb[:, 0, :], ot[0:65, 0, :])
            nc.scalar.activation(
                osb[:, 1, :], ot[0:65, 1, :],
                func=mybir.ActivationFunctionType.Copy,
            )
            nc.sync.dma_start(out=out_d[2 * g, :, qb, :], in_=osb[:, 0, :])
            eng = nc.scalar if last else nc.sync
            eng.dma_start(out=out_d[2 * g + 1, :, qb, :], in_=osb[:, 1, :])

        # ---- schedule ----
        # qb ASCENDING: unit (0, g0) needs only x block 0 (K chunk 0, Q
        # chunk 0, V tiles 0-3), so the exp stream starts as soon as the
        # first quarter of x^T lands.  Each unit's fillers project what the
        # NEXT units need, in x-arrival order.  An 8-warm bridge covers the
        # wq DMA wait.
        proj_qk(kT, wkT, 0, 0)
        for _ in range(2):
            nc.tensor.matmul(warm_ps[:], warm_w[:], warm_x[:], start=True, stop=True)
        proj_qk(qT, wqT, 0, 0)

        def F(*items):
            out = []
            for it in items:
                if it[0] == "v":
                    out.append(lambda t=it[1]: proj_v(t))
                elif it[0] == "k":
                    out.append(lambda g=it[1], c=it[2]: proj_qk(kT, wkT, g, c))
                else:
                    out.append(lambda g=it[1], c=it[2]: proj_qk(qT, wqT, g, c))
            return out

        fill_00 = {
            0: F(("v", 0)),
            1: F(("v", 1)),
            2: F(("v", 2), ("k", 1, 0)),
            3: F(("v", 3), ("q", 1, 0)),
        }
        fill_01 = {
            1: F(("k", 0, 1)),
            2: F(("q", 0, 1)),
            3: F(("v", 4)),
        }
        fill_10 = {
            0: F(("v", 5)),
            1: F(("k", 1, 1)),
            2: F(("q", 1, 1)),
            3: F(("v", 6)),
            4: F(("v", 7)),
            5: F(("k", 0, 2)),
            6: F(("v", 8)),
            7: F(("v", 9)),
        }
        fill_11 = {
            0: F(("q", 0, 2)),
            1: F(("v", 10)),
            2: F(("v", 11)),
            4: F(("q", 1, 2)),
            5: F(("k", 0, 3)),
            6: F(("v", 12)),
            7: F(("v", 13)),
        }
        fill_20 = {
            0: F(("v", 14)),
            1: F(("v", 15)),
            2: F(("q", 0, 3)),
        }
        attn_unit(0, 0, fillers=fill_00)
        attn_unit(0, 1, fillers=fill_01)
        attn_unit(1, 0, fillers=fill_10)
        attn_unit(1, 1, fillers=fill_11)
        attn_unit(2, 0, fillers=fill_20)
        # the last projections are saved as PREFILL for the otherwise
        # fillerless late units (see attn_unit)
        attn_unit(2, 1, prefill=F(("k", 1, 2)))
        attn_unit(3, 0, prefill=F(("k", 1, 3)))
        attn_unit(3, 1, prefill=F(("q", 1, 3)), last=True)

    nc.finalize()
    return nc


def _get_nc():
    if "nc" not in _CACHE:
        _CACHE["nc"] = _build_nc()
    return _CACHE["nc"]


def _make_cmask():
    # triangle: mask[p, f] = 1.0 iff p <= f
    p = np.arange(128)[:, None]
    f = np.arange(128)[None, :]
    return (p <= f).astype(ml_dtypes.bfloat16)


def _make_in_maps(x, Wq, Wk, Wv):
    bf = ml_dtypes.bfloat16
    cmask = _make_cmask()
    in_maps = []
    for c in range(N_CORES):
        b, hg = c // 2, c % 2
        r0 = hg * OUTW
        in_maps.append({
            "xt": np.ascontiguousarray(x[b].T).astype(bf),
            "wqt": np.ascontiguousarray(Wq[r0:r0 + OUTW].T).astype(bf),
            "wkt": np.ascontiguousarray(Wk[r0:r0 + OUTW].T).astype(bf),
            "wvt": np.ascontiguousarray(Wv[r0:r0 + OUTW].T).astype(bf),
            "cmask": cmask,
        })
    return in_maps


def _postprocess(results, B):
    """Host-side unshard: divide unnormalized O^T by the denominator row
    and transpose to natural [T, D] layout."""
    out = np.empty((B, T, D), dtype=np.float32)
    for c in range(N_CORES):
        b, hg = c // 2, c % 2
        r = results[c]["out"].astype(np.float32)  # [HG, 65, NQB, QB]
        r = r.reshape(HG, 65, T)
        for h in range(HG):
            blk = r[h, :64] / r[h, 64:65]  # [64, T]
            out[b, :, hg * OUTW + h * DH:hg * OUTW + (h + 1) * DH] = blk.T
    return out


def kernel(x, Wq, Wk, Wv):
    from concourse.bass_utils import run_bass_kernel_spmd

    nc = _get_nc()
    in_maps = _make_in_maps(x, Wq, Wk, Wv)
    res = run_bass_kernel_spmd(nc, in_maps, core_ids=list(range(N_CORES)))
    return _postprocess(res.results, x.shape[0])


# revision 25
# speedup vs baseline: 1.0710x; 1.0710x over previous
"""Multi-head causal attention (B=4, T=2048, D=512, H=8) on 8 TRN2 NeuronCores.

Sharding: core c handles batch b = c//2 and head-group hg = c%2 (4 heads,
256 output dims).  No collectives needed — 8 fully independent problems.

Per-core algorithm (matmul inputs bf16, O^T accumulation f32 in PSUM):
  - host passes x^T (D,T) and W^T slices (D, 256) in bf16 + a [128,128]
    triangular causal mask
  - Q^T,K^T projections:  qT[dh2,T] = W2h @ xT, two heads stacked per tile
    (head 2g at partitions 0-63, head 2g+1 at partitions 64-127)
  - V projection into augmented-V tiles [k-tile 128, 65] (ones column
    appended -> the O^T matmul also produces the softmax denominator row)
  - flash-style over head-PAIRS: for each (q-block, pair g), per k-tile the
    two heads' score matmuls S^T[k,q] = K^T.T @ Q^T are row-tiled
    (tile_position (0,0) and (64,0) via base_partition auto-derive) and run
    CONCURRENTLY on the PE — K=64 each, so the pair streams in the time of
    one matmul.  exp via ACT (scale=1/8 folded; no max subtraction:
    |scores| < ~4), causal via per-k-tile width restriction + triangle-mask
    multiply on the boundary block.
  - O^T accumulated in PSUM over k-tiles (start/stop groups), software
    pipelined one batch deep: AV(kt-1) is emitted after scores(kt) so the
    PE never stalls on exp(kt).
  - epilogue per unit: single DVE cast of [65, 2, 512] O^T+denominator to
    bf16, DMA'd out UNNORMALIZED; the host divides by the denominator row
    and transposes to natural layout (removes all PE transposes and DVE
    normalize work from the device).

Scheduling (program order == Tile priority): warm-up matmul bursts cover
the x^T DMA shadow (HAM), minimal projection prologue (K g0 chunk 0 +
8-warm bridge + Q g0 chunk 3 + V tile 0), then units (qb DESCENDING,
g=0 then 1) with the remaining projection/V units woven between batches
as PE filler in arrival order.  x^T is DMA'd as 16 column-block pieces in
consumption order across the SP/ACT/gpsimd DGE queues; outputs stream per
unit on the SP queue (last unit split SP/ACT to shorten the tail).
"""

import numpy as np
import ml_dtypes

T = 2048
D = 512
HG = 4  # heads per core
DH = 64
OUTW = HG * DH  # 256
QB = 512  # q block (columns of S^T tiles)
NQB = T // QB  # 4
NKT = T // 128  # 16 k-tiles
N_CORES = 8

_CACHE = {}


def _build_nc():
    import concourse.bacc as bacc
    import concourse.tile as tile
    import concourse.mybir as mybir
    from contextlib import ExitStack

    fp32 = mybir.dt.float32
    bf16 = mybir.dt.bfloat16
    EXP = mybir.ActivationFunctionType.Exp

    nc = bacc.Bacc(None, target_bir_lowering=False)

    xt_d = nc.declare_dram_parameter("xt", [D, T], bf16, isOutput=False)
    wqt_d = nc.declare_dram_parameter("wqt", [D, OUTW], bf16, isOutput=False)
    wkt_d = nc.declare_dram_parameter("wkt", [D, OUTW], bf16, isOutput=False)
    wvt_d = nc.declare_dram_parameter("wvt", [D, OUTW], bf16, isOutput=False)
    cmask_d = nc.declare_dram_parameter("cmask", [128, 128], bf16, isOutput=False)
    # unnormalized O^T + denominator row: [head, 65, qb, 512]
    out_d = nc.declare_dram_parameter("out", [HG, 65, NQB, QB], bf16, isOutput=True)

    with tile.TileContext(nc) as tc, ExitStack() as ctx:
        const = ctx.enter_context(tc.tile_pool(name="const", bufs=1))
        ps_s = ctx.enter_context(tc.tile_pool(name="ps_s", bufs=2, space="PSUM"))
        pt_pool = ctx.enter_context(tc.tile_pool(name="pt", bufs=4))
        osb_pool = ctx.enter_context(tc.tile_pool(name="osb", bufs=2))

        # ---- input loads ----
        # x^T arrives as 16 column-block pieces (chunk c x q-block b), DMA'd
        # in ascending consumption order (block 0 gates the prologue
        # projections) and spread over the three DGE queues.  The scalar
        # (ACT) queue only carries pieces that finish BEFORE the first exp,
        # so triggers never steal ACT time from the exp stream.
        xTb = [[const.tile([128, QB], bf16, tag=f"xT{c}_{b}", name=f"xT{c}_{b}")
                for b in range(4)] for c in range(4)]
        wkT = [const.tile([128, OUTW], bf16, tag=f"wkT{c}", name=f"wkT{c}")
               for c in range(4)]
        wqT = [const.tile([128, OUTW], bf16, tag=f"wqT{c}", name=f"wqT{c}")
               for c in range(4)]
        wvT = [const.tile([128, OUTW], bf16, tag=f"wvT{c}", name=f"wvT{c}")
               for c in range(4)]
        mask_sb = const.tile([128, 128], bf16, name="mask_sb")

        def ld_w(eng, wt, dram, c):
            eng.dma_start(out=wt[c][:], in_=dram[c * 128:(c + 1) * 128, :])

        def ld_x(eng, c, b):
            eng.dma_start(
                out=xTb[c][b][:],
                in_=xt_d[c * 128:(c + 1) * 128, b * QB:(b + 1) * QB],
            )

        ld_w(nc.sync, wkT, wkt_d, 0)
        ld_w(nc.sync, wkT, wkt_d, 1)
        ld_w(nc.sync, wkT, wkt_d, 3)
        ld_x(nc.sync, 0, 0)
        ld_w(nc.sync, wqT, wqt_d, 0)
        ld_w(nc.sync, wqT, wqt_d, 1)
        ld_x(nc.sync, 0, 1)
        ld_x(nc.sync, 1, 1)
        ld_x(nc.sync, 2, 1)
        ld_x(nc.sync, 3, 1)
        ld_x(nc.sync, 0, 3)
        ld_x(nc.sync, 1, 3)

        ld_w(nc.scalar, wkT, wkt_d, 2)
        ld_x(nc.scalar, 2, 0)
        ld_x(nc.scalar, 1, 0)
        ld_w(nc.scalar, wqT, wqt_d, 2)
        ld_w(nc.scalar, wqT, wqt_d, 3)
        nc.scalar.dma_start(out=mask_sb[:], in_=cmask_d[:])

        # slow SWDGE triggers (~1us each) only for the one block-0 piece
        # that balances the fast queues, plus late-needed pieces
        ld_x(nc.gpsimd, 3, 0)
        for c in range(4):
            ld_w(nc.gpsimd, wvT, wvt_d, c)
        ld_x(nc.gpsimd, 0, 2)
        ld_x(nc.gpsimd, 1, 2)
        ld_x(nc.gpsimd, 2, 2)
        ld_x(nc.gpsimd, 3, 2)
        ld_x(nc.gpsimd, 2, 3)
        ld_x(nc.gpsimd, 3, 3)

        # ---- HAM warm-up burst ----
        # The PE clock needs a fully-busy window to ramp.  Burn the x^T DMA
        # shadow with dense dummy matmuls so real work starts warm.
        warm_w = const.tile([128, 128], bf16, name="warm_w")
        warm_x = const.tile([128, QB], bf16, name="warm_x")
        nc.vector.memset(warm_w[:], 0.5)
        nc.vector.memset(warm_x[:], 0.5)
        # 12 warms ≈ the ~6.5us wk/wq/x-block-0 DMA window (cold 512ns each,
        # ~256ns once the ramp trips mid-burst) — sized so the PE never idles
        # a full MID window before the first projection.
        warm_ps = ps_s.tile([128, QB], fp32, tag="ps", name="warm_ps")
        for _ in range(12):
            nc.tensor.matmul(warm_ps[:], warm_w[:], warm_x[:], start=True, stop=True)

        # ---- persistent SBUF tensors ----
        qT = [const.tile([128, T], bf16, tag=f"qT{g}", name=f"qT{g}") for g in range(2)]
        kT = [const.tile([128, T], bf16, tag=f"kT{g}", name=f"kT{g}") for g in range(2)]
        vaug = const.tile([128, NKT, HG, 65], bf16, name="vaug")
        nc.vector.memset(vaug[:, :, :, 64:65], 1.0)

        def proj_qk(dst, wt, g, qb4):
            ps = ps_s.tile([128, QB], fp32, tag="ps", name="ps")
            for c in range(4):
                nc.tensor.matmul(
                    ps[:],
                    wt[c][:, g * 128:(g + 1) * 128],
                    xTb[c][qb4][:],
                    start=(c == 0),
                    stop=(c == 3),
                )
            nc.vector.tensor_copy(dst[g][:, qb4 * QB:(qb4 + 1) * QB], ps[:])

        def proj_v(tt):
            ps = ps_s.tile([128, OUTW], fp32, tag="ps", name="ps")
            for c in range(4):
                nc.tensor.matmul(
                    ps[:],
                    xTb[c][tt // 4][:, (tt % 4) * 128:(tt % 4 + 1) * 128],
                    wvT[c][:, 0:OUTW],
                    start=(c == 0),
                    stop=(c == 3),
                )
            nc.vector.tensor_copy(
                vaug[:, tt, :, 0:64],
                ps[:].rearrange("p (h d) -> p h d", h=HG),
            )

        def attn_unit(qb, g, fillers=None, prefill=None, last=False):
            """One (q-block, head-pair) attention unit.  The two heads'
            score matmuls per k-tile are row-tiled (partitions 0-63 /
            64-127) and run concurrently on the PE.  AV(kt-1) is emitted
            after scores(kt) (1-deep software pipeline) so the PE works
            through the exp latency.  fillers[j] = list of closures run
            as PE filler after batch j's scores."""
            nkt = qb * 4 + 4
            # prefill runs BEFORE the first scores: at a unit boundary the
            # first scores block on the st-slot WAR until ACT drains the
            # previous unit's diagonal-exp backlog, and the in-order PE
            # queue would stall fillers emitted behind them.
            if prefill:
                for f in prefill:
                    f()
            ot = ps_s.tile([128, 2, QB], fp32, tag="ot", bufs=1, name="ot")
            osb = osb_pool.tile([65, 2, QB], bf16, tag="osb", name="osb")
            pend = []  # (kt, pt, q0, width) awaiting AV matmuls (lag 2)

            def emit_av(kt, pt, q0, width):
                for i in range(2):
                    nc.tensor.matmul(
                        ot[0:65, i, q0:q0 + width],
                        vaug[:, kt, 2 * g + i, :],
                        pt[:, i, q0:q0 + width],
                        start=(kt == 0),
                        stop=(kt == nkt - 1),
                    )
                # O^T columns [0,256) are final after diagonal kt qb*4+1,
                # columns [256,512) after the last kt: cast each half as
                # soon as it is final (subtile deps) so most of the cast is
                # off the unit boundary and the ot PSUM slot frees early.
                if kt == qb * 4 + 1:
                    nc.vector.tensor_copy(osb[:, :, 0:256], ot[0:65, :, 0:256])
                elif kt == nkt - 1:
                    nc.vector.tensor_copy(osb[:, :, 256:QB],
                                          ot[0:65, :, 256:QB])

            for kt in range(nkt):
                diag = kt >= qb * 4
                q0 = (kt - qb * 4) * 128 if diag else 0
                width = QB - q0
                st = ps_s.tile([128, 2, QB], fp32, tag="st", name="st")
                for i in range(2):
                    nc.tensor.matmul(
                        st[:, i, q0:q0 + width],
                        kT[g][64 * i:64 * i + 64, kt * 128:(kt + 1) * 128],
                        qT[g][64 * i:64 * i + 64,
                              qb * QB + q0:qb * QB + q0 + width],
                        start=True,
                        stop=True,
                    )
                if fillers:
                    for f in fillers.get(kt, ()):
                        f()
                # AV lags 2 batches so the unit's first AV (which waits on
                # the previous unit's ot WAR) issues behind two score pairs.
                if len(pend) == 2:
                    emit_av(*pend.pop(0))
                pt = pt_pool.tile([128, 2, QB], bf16, tag="pt", name="pt")
                nc.scalar.activation(
                    pt[:, :, q0:q0 + width], st[:, :, q0:q0 + width],
                    func=EXP, scale=0.125,
                )
                if diag:
                    for i in range(2):
                        nc.vector.tensor_mul(
                            pt[:, i, q0:q0 + 128], pt[:, i, q0:q0 + 128],
                            mask_sb[:],
                        )
                pend.append((kt, pt, q0, width))
            for p in pend:
                emit_av(*p)

            # stream out unnormalized O^T + denominator row (host divides
            # + transposes); the casts already happened in emit_av.
            nc.sync.dma_start(out=out_d[2 * g, :, qb, :], in_=osb[:, 0, :])
            eng = nc.scalar if last else nc.sync
            eng.dma_start(out=out_d[2 * g + 1, :, qb, :], in_=osb[:, 1, :])

        # ---- schedule ----
        # qb ASCENDING: unit (0, g0) needs only x block 0 (K chunk 0, Q
        # chunk 0, V tiles 0-3), so the exp stream starts as soon as the
        # first quarter of x^T lands.  Each unit's fillers project what the
        # NEXT units need, in x-arrival order.  An 8-warm bridge covers the
        # wq DMA wait.
        proj_qk(kT, wkT, 0, 0)
        for _ in range(2):
            nc.tensor.matmul(warm_ps[:], warm_w[:], warm_x[:], start=True, stop=True)
        proj_qk(qT, wqT, 0, 0)

        def F(*items):
            out = []
            for it in items:
                if it[0] == "v":
                    out.append(lambda t=it[1]: proj_v(t))
                elif it[0] == "k":
                    out.append(lambda g=it[1], c=it[2]: proj_qk(kT, wkT, g, c))
                else:
                    out.append(lambda g=it[1], c=it[2]: proj_qk(qT, wqT, g, c))
            return out

        fill_00 = {
            0: F(("v", 0)),
            1: F(("v", 1)),
            2: F(("v", 2), ("k", 1, 0)),
            3: F(("v", 3), ("q", 1, 0)),
        }
        fill_01 = {
            1: F(("k", 0, 1)),
            2: F(("q", 0, 1)),
            3: F(("v", 4)),
        }
        fill_10 = {
            0: F(("v", 5)),
            1: F(("k", 1, 1)),
            2: F(("q", 1, 1)),
            3: F(("v", 6)),
            4: F(("v", 7)),
            5: F(("k", 0, 2)),
            6: F(("v", 8)),
            7: F(("v", 9)),
        }
        fill_11 = {
            0: F(("q", 0, 2)),
            1: F(("v", 10)),
            2: F(("v", 11)),
            4: F(("q", 1, 2)),
            5: F(("k", 0, 3)),
            6: F(("v", 12)),
            7: F(("v", 13)),
        }
        fill_20 = {
            0: F(("v", 14)),
            1: F(("v", 15)),
            2: F(("q", 0, 3)),
        }
        attn_unit(0, 0, fillers=fill_00)
        attn_unit(0, 1, fillers=fill_01)
        attn_unit(1, 0, fillers=fill_10)
        attn_unit(1, 1, fillers=fill_11)
        attn_unit(2, 0, fillers=fill_20)
        # the last projections are saved as PREFILL for the otherwise
        # fillerless late units (see attn_unit)
        attn_unit(2, 1, prefill=F(("k", 1, 2)))
        attn_unit(3, 0, prefill=F(("k", 1, 3)))
        attn_unit(3, 1, prefill=F(("q", 1, 3)), last=True)

    nc.finalize()
    return nc


def _get_nc():
    if "nc" not in _CACHE:
        _CACHE["nc"] = _build_nc()
    return _CACHE["nc"]


def _make_cmask():
    # triangle: mask[p, f] = 1.0 iff p <= f
    p = np.arange(128)[:, None]
    f = np.arange(128)[None, :]
    return (p <= f).astype(ml_dtypes.bfloat16)


def _make_in_maps(x, Wq, Wk, Wv):
    bf = ml_dtypes.bfloat16
    cmask = _make_cmask()
    in_maps = []
    for c in range(N_CORES):
        b, hg = c // 2, c % 2
        r0 = hg * OUTW
        in_maps.append({
            "xt": np.ascontiguousarray(x[b].T).astype(bf),
            "wqt": np.ascontiguousarray(Wq[r0:r0 + OUTW].T).astype(bf),
            "wkt": np.ascontiguousarray(Wk[r0:r0 + OUTW].T).astype(bf),
            "wvt": np.ascontiguousarray(Wv[r0:r0 + OUTW].T).astype(bf),
            "cmask": cmask,
        })
    return in_maps


def _postprocess(results, B):
    """Host-side unshard: divide unnormalized O^T by the denominator row
    and transpose to natural [T, D] layout."""
    out = np.empty((B, T, D), dtype=np.float32)
    for c in range(N_CORES):
        b, hg = c // 2, c % 2
        r = results[c]["out"].astype(np.float32)  # [HG, 65, NQB, QB]
        r = r.reshape(HG, 65, T)
        for h in range(HG):
            blk = r[h, :64] / r[h, 64:65]  # [64, T]
            out[b, :, hg * OUTW + h * DH:hg * OUTW + (h + 1) * DH] = blk.T
    return out


def kernel(x, Wq, Wk, Wv):
    from concourse.bass_utils import run_bass_kernel_spmd

    nc = _get_nc()
    in_maps = _make_in_maps(x, Wq, Wk, Wv)
    res = run_bass_kernel_spmd(nc, in_maps, core_ids=list(range(N_CORES)))
    return _postprocess(res.results, x.shape[0])
